# revision 29
# baseline (speedup 1.0000x reference)
"""Self-attention (CrossAttention module with q=k=v=x) kernel for Trainium2.

Problem: x [B=4, N=4096, H=256] fp32; Wq/Wk/Wv [256,256], bq/bk/bv [256].
  q = x@Wq.T+bq ; k = x@Wk.T+bk ; v = x@Wv.T+bv
  out = softmax(q@k.T) @ v          (no 1/sqrt(d) scaling)

Sharding: 8 cores = batch (4) x query-halves (2). Each core holds the full
K/V sequence for its batch element and 2048 query rows.

Per-core layout strategy (fp32 accumulation in PSUM throughout):
  - Projections + scores matmuls run in fp16 (same 11-bit mantissa as
    TF32 for this data range, but FWL-eligible so LDWEIGHTS is ~2x
    faster); exp(S) and V run in bf16 (fp32-like exponent range, so no
    per-row max subtraction is needed for the softmax).
  - Host pre-transposes x -> xT [H, N] (fp16) and weights -> W.T [h, o]
    so Q^T/K^T land in [o, n] layout directly off the projection matmuls.
  - scores are computed TRANSPOSED: S_T[j, i] = sum_o K_T[o,j] * Q_T[o,i]
    (K_T chunk stationary, Q_T moving), so exp(S_T) feeds the AV matmul
    as the stationary operand with no on-chip transpose.  Query blocks
    are processed in pairs sharing each kT stationary load.
  - V is produced in natural [n, o] layout with two trailing columns of
    ones: att_psum[i, H] = row sum of P -- the softmax denominator comes
    free off the AV matmul.
  - normalize with DVE reciprocal + per-partition broadcast multiply.
  - a short junk-matmul warmup keeps the PE busy (HAM clock at 2.4 GHz)
    while x streams in.
  - v-bias is added on the host after gathering (softmax rows sum to 1).
"""

import sys

import numpy as np

if "/opt/trn_rl_repo" not in sys.path:
    sys.path.insert(0, "/opt/trn_rl_repo")

B, N, H = 4, 4096, 256
P = 128
NQ = N // 2  # query rows per core


def build_nc(nkv=N, nq=NQ, iblk=512, salt=0):
    import concourse.mybir as mybir
    import concourse.tile as tile
    from concourse import bacc

    f32 = mybir.dt.float32
    FR = mybir.dt.float16   # projections + scores matmul dtype (11-bit mantissa)
    AVT = mybir.dt.bfloat16  # exp(S) and V dtype: needs fp32-like range
    Exp = mybir.ActivationFunctionType.Exp

    assert nkv % 512 == 0 and nq % iblk == 0 and iblk % P == 0 and iblk <= 512
    JC = nkv // P          # key chunks
    ICH = iblk // P        # query sub-chunks per block
    NBLK = nq // iblk      # query blocks
    kseg = 512             # K_T projection segment (moving free dim)
    qseg = min(512, nq)

    nc = bacc.Bacc("TRN2", target_bir_lowering=False, debug=False)

    xT_d = nc.dram_tensor("xT", [H, nkv], FR, kind="ExternalInput").ap()
    xqT_d = nc.dram_tensor("xqT", [H, nq], FR, kind="ExternalInput").ap()
    wa_d = nc.dram_tensor("wa", [H, H], FR, kind="ExternalInput").ap()
    wvT_d = nc.dram_tensor("wvT", [H, H], FR, kind="ExternalInput").ap()
    wsc_d = nc.dram_tensor("wsc", [P, nkv // P], f32, kind="ExternalInput").ap()
    att_d = nc.dram_tensor("att", [nq, H], f32, kind="ExternalOutput").ap()
    warm_d = nc.dram_tensor("warm", [P, 2], f32, kind="ExternalOutput").ap()

    with tile.TileContext(nc) as tc:
        with tc.tile_pool(name="io", bufs=1) as io, \
             tc.tile_pool(name="kqv", bufs=1) as kqv, \
             tc.tile_pool(name="expp", bufs=JC + 4) as expp, \
             tc.tile_pool(name="op", bufs=4) as op, \
             tc.tile_pool(name="psmm", bufs=2, space="PSUM") as psmm, \
             tc.tile_pool(name="psatt", bufs=4, space="PSUM") as psatt:

            # ---- input loads (gpsimd DMA casts f32 -> f32r on the fly) ----
            xt = [io.tile([P, nkv], FR, tag=f"xt{h}", name=f"xt{h}") for h in range(2)]
            xq = [io.tile([P, nq], FR, tag=f"xq{h}", name=f"xq{h}") for h in range(2)]
            wq = [io.tile([P, H], FR, tag=f"wq{h}", name=f"wq{h}") for h in range(2)]
            wv = [io.tile([P, H], FR, tag=f"wv{h}", name=f"wv{h}") for h in range(2)]
            wsc = io.tile([P, JC], f32, tag="wsc", name="wsc")
            # weights/biases first (small), then x in 1024-column chunks so
            # the projection matmuls can start before the full x has landed
            # (Tile tracks deps per dma_start instruction).
            for h in range(2):
                hs = slice(h * P, (h + 1) * P)
                nc.sync.dma_start(wq[h][:], wa_d[hs, :])
                nc.sync.dma_start(wv[h][:], wvT_d[hs, :])
            nc.sync.dma_start(wsc[:], wsc_d[:])
            xchunk = 1024
            nxq = (nq + xchunk - 1) // xchunk
            nxt = (nkv + xchunk - 1) // xchunk
            for c in range(max(nxq, nxt)):
                if c < nxq:
                    cs = slice(c * xchunk, min((c + 1) * xchunk, nq))
                    for h in range(2):
                        hs = slice(h * P, (h + 1) * P)
                        nc.sync.dma_start(xq[h][:, cs], xqT_d[hs, cs])
                if c < nxt:
                    cs = slice(c * xchunk, min((c + 1) * xchunk, nkv))
                    for h in range(2):
                        hs = slice(h * P, (h + 1) * P)
                        nc.sync.dma_start(xt[h][:, cs], xT_d[hs, cs])

            # ---- PE warm-up ----
            # ~10 junk matmuls on the (tiny, early-arriving) weight tiles
            # keep the PE busy while x streams in, so the HAM clock gate
            # reaches 2.4 GHz before the real work starts.  The result is
            # written to a throwaway output so DCE keeps the chain.
            wrm = io.tile([P, H], FR, tag="wrm", name="wrm")
            nc.vector.memset(wrm[:], 0.5)
            wps = psatt.tile([P, H], f32, tag="att", name="wps")
            nwarm = 18 + salt
            for r in range(nwarm):
                nc.tensor.matmul(wps[:], wrm[:, 0:P], wrm[:],
                                 start=(r == 0), stop=(r == nwarm - 1))
            wsb = op.tile([P, 2], f32, tag="wsb", name="wsb")
            nc.vector.tensor_copy(wsb[:], wps[:, 0:2])
            nc.sync.dma_start(warm_d[:], wsb[:])

            # ---- projections ----
            # Q_T first (xq arrives first), then K_T / V interleaved in
            # xT-chunk arrival order.
            qT = [kqv.tile([P, nq], FR, tag=f"qT{oc}", name=f"qT{oc}") for oc in range(2)]
            ones2 = io.tile([P, 2], f32, tag="ones2", name="ones2")
            nc.vector.memset(ones2[:], 1.0)
            vt = [kqv.tile([P, H + 2], AVT, tag=f"v{j}", name=f"v{j}") for j in range(JC)]

            # Q/K/V projections emitted in x-chunk arrival order:
            # Q(c) then K(c)+V(c) per 1024-column chunk.
            # K_T[o, j] = sum_h WkT[h, o] * xT[h, j]   (+ bk[o]);
            # V[n, o] = sum_h xT[h, n] * WvT[h, o] (no bias: added on host),
            # plus two trailing columns of ones: column H yields the softmax
            # denominator straight off the AV matmul.
            for c in range(max(nxq, nxt)):
                if c < nxq:
                    c0, c1 = c * xchunk, min((c + 1) * xchunk, nq)
                    for s in range(c0 // qseg, c1 // qseg):
                        ss = slice(s * qseg, (s + 1) * qseg)
                        for oc in range(2):
                            ocs = slice(oc * P, (oc + 1) * P)
                            pq = psatt.tile([P, qseg], f32, tag="att", name="pq")
                            for h in range(2):
                                nc.tensor.matmul(pq[:], wq[h][:, ocs], xq[h][:, ss],
                                                 start=(h == 0), stop=(h == 1))
                            nc.vector.tensor_copy(qT[oc][:, ss], pq[:])
                if c < nxt:
                    c0, c1 = c * xchunk, min((c + 1) * xchunk, nkv)
                    for j in range(c0 // P, c1 // P):
                        js = slice(j * P, (j + 1) * P)
                        pv = psatt.tile([P, H], f32, tag="att", name="pv")
                        for h in range(2):
                            nc.tensor.matmul(pv[:], xt[h][:, js], wv[h][:],
                                             start=(h == 0), stop=(h == 1))
                        nc.vector.tensor_copy(vt[j][:, 0:H], pv[:])
                        nc.vector.tensor_copy(vt[j][:, H:H + 2], ones2[:])

            # ---- attention blocks ----
            # Blocks are processed in pairs: the scores matmuls for both
            # blocks of a pair share each kT stationary load (halving the
            # scores LDWEIGHTS count, which is serialized with the matmuls
            # on the PE).  Block b0's AV runs inline per key-chunk; block
            # b1's exp(S) tiles are buffered in SBUF and consumed in a
            # second AV sweep (PSUM can only hold one block's accumulators
            # plus the rotating scores tiles).
            def av_sweep(att_ps, exs, blk):
                # ic-major: each att accumulator closes after its own 1/ICH
                # of the sweep, so normalize + DMA-out overlap the rest.
                for ic in range(ICH):
                    ics = slice(ic * P, (ic + 1) * P)
                    for jc in range(JC):
                        nc.tensor.matmul(att_ps[ic][:], exs[jc][:, ics],
                                         vt[jc][:],
                                         start=(jc == 0), stop=(jc == JC - 1))

            def normalize_one(att_tile, blk, ic):
                rec = op.tile([P, 1], f32, tag="rec", name="rec")
                nc.vector.reciprocal(rec[:], att_tile[:, H:H + 1])
                ao = op.tile([P, H], f32, tag="ao", name="ao")
                nc.vector.tensor_scalar_mul(ao[:], att_tile[:, 0:H], rec[:])
                r0 = blk * iblk + ic * P
                nc.sync.dma_start(att_d[r0:r0 + P, :], ao[:])

            def normalize(att_ps, blk):
                for ic in range(ICH):
                    normalize_one(att_ps[ic], blk, ic)

            blk = 0
            GRP = 2
            while blk < NBLK:
                g = min(GRP, NBLK - blk)  # blocks in this group
                bss = [slice((blk + b) * iblk, (blk + b + 1) * iblk)
                       for b in range(g)]
                att_ps = [psatt.tile([P, H + 2], f32, tag="att", name="attps")
                          for _ in range(ICH)]
                exs = [[] for _ in range(g)]

                def av_b0(jc):
                    for ic in range(ICH):
                        ics = slice(ic * P, (ic + 1) * P)
                        nc.tensor.matmul(att_ps[ic][:], exs[0][jc][:, ics],
                                         vt[jc][:],
                                         start=(jc == 0), stop=(jc == JC - 1))

                for jc in range(JC):
                    jcs = slice(jc * P, (jc + 1) * P)
                    # one PSUM tile spanning g banks; each block's scores
                    # matmuls stay inside their own bank, and a single
                    # ACTIVATE exps the whole tile (halves ACT's per-op
                    # overhead, which was pacing the pair phases).
                    scd = psmm.tile([P, g * iblk], f32, tag="scd", name="scd")
                    for oc in range(2):
                        for b in range(g):
                            nc.tensor.matmul(scd[:, b * iblk:(b + 1) * iblk],
                                             xt[oc][:, jcs],
                                             qT[oc][:, bss[b]],
                                             start=(oc == 0), stop=(oc == 1))
                    ex2 = expp.tile([P, g * iblk], AVT, tag="ex", name="ex2")
                    nc.scalar.activation(ex2[:], scd[:], Exp,
                                         bias=wsc[:, jc:jc + 1])
                    for b in range(g):
                        exs[b].append(ex2[:, b * iblk:(b + 1) * iblk])
                    av_b0(jc)
                normalize(att_ps, blk)
                for b in range(1, g):
                    att_psb = [psatt.tile([P, H + 2], f32, tag="att",
                                          name=f"attps{b}")
                               for _ in range(ICH)]
                    for ic in range(ICH):
                        ics = slice(ic * P, (ic + 1) * P)
                        for jc in range(JC):
                            nc.tensor.matmul(att_psb[ic][:],
                                             exs[b][jc][:, ics], vt[jc][:],
                                             start=(jc == 0),
                                             stop=(jc == JC - 1))
                        normalize_one(att_psb[ic], blk + b, ic)
                blk += g

    nc.compile()
    return nc


_NC_CACHE = {}


def _get_nc(nkv=N, nq=NQ, iblk=512):
    key = (nkv, nq, iblk)
    if key not in _NC_CACHE:
        _NC_CACHE[key] = build_nc(*key)
    return _NC_CACHE[key]


def _make_in_maps(x, Wq, bq, Wk, bk, Wv):
    # scores algebra: q_i.k_j = x_i (Wq^T Wk) x_j^T + u_i + w_j + c where
    # u_i and c are constant per softmax row (dropped -- softmax-invariant)
    # and w_j = x_j . (Wk^T bq) is applied as the exp() bias on-device.
    wa = np.ascontiguousarray(
        (Wq.T.astype(np.float64) @ Wk.astype(np.float64)).astype(np.float16))
    wvT = np.ascontiguousarray(Wv.T.astype(np.float16))
    wkbq = Wk.T.astype(np.float64) @ bq.astype(np.float64)
    x16 = x.astype(np.float16)
    xT = [np.ascontiguousarray(x16[b].T) for b in range(B)]
    wsc = [np.ascontiguousarray(
        (x[b].astype(np.float64) @ wkbq).astype(np.float32)
        .reshape(N // 128, 128).T) for b in range(B)]
    in_maps = []
    for c in range(8):
        b, half = c // 2, c % 2
        in_maps.append({
            "xT": xT[b],
            "xqT": np.ascontiguousarray(x16[b, half * NQ:(half + 1) * NQ, :].T),
            "wa": wa, "wvT": wvT, "wsc": wsc[b],
        })
    return in_maps


def _run(inputs, trace=False):
    from concourse.bass_utils import run_bass_kernel_spmd

    x = np.asarray(inputs["x"], dtype=np.float32)
    Wq = np.asarray(inputs["Wq"], dtype=np.float32)
    bq = np.asarray(inputs["bq"], dtype=np.float32)
    Wk = np.asarray(inputs["Wk"], dtype=np.float32)
    bk = np.asarray(inputs["bk"], dtype=np.float32)
    Wv = np.asarray(inputs["Wv"], dtype=np.float32)
    bv = np.asarray(inputs["bv"], dtype=np.float32)

    in_maps = _make_in_maps(x, Wq, bq, Wk, bk, Wv)
    # The device occasionally wedges on the first execution of a fresh
    # NEFF (NRT_EXEC_UNIT_UNRECOVERABLE); a retry with a slightly
    # perturbed program (different walrus schedule) recovers.
    last_exc = None
    for attempt in range(3):
        try:
            nc = _get_nc() if attempt == 0 else build_nc(salt=attempt)
            res = run_bass_kernel_spmd(nc, in_maps, list(range(8)), trace=trace)
            break
        except Exception as e:  # noqa: BLE001
            last_exc = e
            import os as _os
            import time as _time
            _os.environ["NEURON_RT_RESET_CORES"] = "1"
            _time.sleep(3)
    else:
        raise last_exc

    out = np.empty((B, N, H), dtype=np.float32)
    for c in range(8):
        b, half = c // 2, c % 2
        out[b, half * NQ:(half + 1) * NQ, :] = res.results[c]["att"] + bv
    return out, res


def kernel(**inputs) -> np.ndarray:
    out, _ = _run(inputs, trace=False)
    return out


# revision 30
# speedup vs baseline: 1.0643x; 1.0643x over previous
"""Self-attention (CrossAttention module with q=k=v=x) kernel for Trainium2.

Problem: x [B=4, N=4096, H=256] fp32; Wq/Wk/Wv [256,256], bq/bk/bv [256].
  q = x@Wq.T+bq ; k = x@Wk.T+bk ; v = x@Wv.T+bv
  out = softmax(q@k.T) @ v          (no 1/sqrt(d) scaling)

Sharding: 8 cores = batch (4) x query-halves (2). Each core holds the full
K/V sequence for its batch element and 2048 query rows.

Per-core layout strategy (fp32 accumulation in PSUM throughout):
  - Projections + scores matmuls run in fp16 (same 11-bit mantissa as
    TF32 for this data range, but FWL-eligible so LDWEIGHTS is ~2x
    faster); exp(S) and V run in bf16 (fp32-like exponent range, so no
    per-row max subtraction is needed for the softmax).
  - Host pre-transposes x -> xT [H, N] (fp16) and weights -> W.T [h, o]
    so Q^T/K^T land in [o, n] layout directly off the projection matmuls.
  - scores are computed TRANSPOSED: S_T[j, i] = sum_o K_T[o,j] * Q_T[o,i]
    (K_T chunk stationary, Q_T moving), so exp(S_T) feeds the AV matmul
    as the stationary operand with no on-chip transpose.  Query blocks
    are processed in pairs sharing each kT stationary load.
  - V is produced in natural [n, o] layout with two trailing columns of
    ones: att_psum[i, H] = row sum of P -- the softmax denominator comes
    free off the AV matmul.
  - normalize with DVE reciprocal + per-partition broadcast multiply.
  - a short junk-matmul warmup keeps the PE busy (HAM clock at 2.4 GHz)
    while x streams in.
  - v-bias is added on the host after gathering (softmax rows sum to 1).
"""

import sys

import numpy as np

if "/opt/trn_rl_repo" not in sys.path:
    sys.path.insert(0, "/opt/trn_rl_repo")

B, N, H = 4, 4096, 256
P = 128
NQ = N // 2  # query rows per core


def build_nc(nkv=N, nq=NQ, iblk=512, salt=0):
    import concourse.mybir as mybir
    import concourse.tile as tile
    from concourse import bacc

    f32 = mybir.dt.float32
    FR = mybir.dt.float16   # projections + scores matmul dtype (11-bit mantissa)
    AVT = mybir.dt.bfloat16  # exp(S) and V dtype: needs fp32-like range
    Exp = mybir.ActivationFunctionType.Exp

    assert nkv % 512 == 0 and nq % iblk == 0 and iblk % P == 0 and iblk <= 512
    JC = nkv // P          # key chunks
    ICH = iblk // P        # query sub-chunks per block
    NBLK = nq // iblk      # query blocks
    kseg = 512             # K_T projection segment (moving free dim)
    qseg = min(512, nq)

    nc = bacc.Bacc("TRN2", target_bir_lowering=False, debug=False)

    xT_d = nc.dram_tensor("xT", [H, nkv], FR, kind="ExternalInput").ap()
    xqT_d = nc.dram_tensor("xqT", [H, nq], FR, kind="ExternalInput").ap()
    wa_d = nc.dram_tensor("wa", [H, H], FR, kind="ExternalInput").ap()
    wvT_d = nc.dram_tensor("wvT", [H, H], FR, kind="ExternalInput").ap()
    wsc_d = nc.dram_tensor("wsc", [P, nkv // P], f32, kind="ExternalInput").ap()
    att_d = nc.dram_tensor("att", [nq, H], f32, kind="ExternalOutput").ap()
    warm_d = nc.dram_tensor("warm", [P, 2], f32, kind="ExternalOutput").ap()

    with tile.TileContext(nc) as tc:
        with tc.tile_pool(name="io", bufs=1) as io, \
             tc.tile_pool(name="kqv", bufs=1) as kqv, \
             tc.tile_pool(name="expp", bufs=JC + 8) as expp, \
             tc.tile_pool(name="op", bufs=4) as op, \
             tc.tile_pool(name="psmm", bufs=3, space="PSUM") as psmm, \
             tc.tile_pool(name="psatt", bufs=5, space="PSUM") as psatt:

            # ---- input loads (gpsimd DMA casts f32 -> f32r on the fly) ----
            xt = [io.tile([P, nkv], FR, tag=f"xt{h}", name=f"xt{h}") for h in range(2)]
            xq = [io.tile([P, nq], FR, tag=f"xq{h}", name=f"xq{h}") for h in range(2)]
            wq = [io.tile([P, H], FR, tag=f"wq{h}", name=f"wq{h}") for h in range(2)]
            wv = [io.tile([P, H], FR, tag=f"wv{h}", name=f"wv{h}") for h in range(2)]
            wsc = io.tile([P, JC], f32, tag="wsc", name="wsc")
            # weights/biases first (small), then x in 1024-column chunks so
            # the projection matmuls can start before the full x has landed
            # (Tile tracks deps per dma_start instruction).
            for h in range(2):
                hs = slice(h * P, (h + 1) * P)
                nc.sync.dma_start(wq[h][:], wa_d[hs, :])
                nc.sync.dma_start(wv[h][:], wvT_d[hs, :])
            nc.sync.dma_start(wsc[:], wsc_d[:])
            xchunk = 1024
            nxq = (nq + xchunk - 1) // xchunk
            nxt = (nkv + xchunk - 1) // xchunk
            for c in range(max(nxq, nxt)):
                if c < nxq:
                    cs = slice(c * xchunk, min((c + 1) * xchunk, nq))
                    for h in range(2):
                        hs = slice(h * P, (h + 1) * P)
                        nc.sync.dma_start(xq[h][:, cs], xqT_d[hs, cs])
                if c < nxt:
                    cs = slice(c * xchunk, min((c + 1) * xchunk, nkv))
                    for h in range(2):
                        hs = slice(h * P, (h + 1) * P)
                        nc.sync.dma_start(xt[h][:, cs], xT_d[hs, cs])

            # ---- PE warm-up ----
            # ~10 junk matmuls on the (tiny, early-arriving) weight tiles
            # keep the PE busy while x streams in, so the HAM clock gate
            # reaches 2.4 GHz before the real work starts.  The result is
            # written to a throwaway output so DCE keeps the chain.
            wrm = io.tile([P, H], FR, tag="wrm", name="wrm")
            nc.vector.memset(wrm[:], 0.5)
            wps = psmm.tile([P, H], f32, tag="mm", name="wps")
            nwarm = 18 + salt
            for r in range(nwarm):
                nc.tensor.matmul(wps[:], wrm[:, 0:P], wrm[:],
                                 start=(r == 0), stop=(r == nwarm - 1))
            wsb = op.tile([P, 2], f32, tag="wsb", name="wsb")
            nc.vector.tensor_copy(wsb[:], wps[:, 0:2])
            nc.sync.dma_start(warm_d[:], wsb[:])

            # ---- projections ----
            # Q_T first (xq arrives first), then K_T / V interleaved in
            # xT-chunk arrival order.
            qT = [kqv.tile([P, nq], FR, tag=f"qT{oc}", name=f"qT{oc}") for oc in range(2)]
            ones2 = io.tile([P, 2], f32, tag="ones2", name="ones2")
            nc.vector.memset(ones2[:], 1.0)
            vt = [kqv.tile([P, H + 2], AVT, tag=f"v{j}", name=f"v{j}") for j in range(JC)]

            # Q/K/V projections emitted in x-chunk arrival order:
            # Q(c) then K(c)+V(c) per 1024-column chunk.
            # K_T[o, j] = sum_h WkT[h, o] * xT[h, j]   (+ bk[o]);
            # V[n, o] = sum_h xT[h, n] * WvT[h, o] (no bias: added on host),
            # plus two trailing columns of ones: column H yields the softmax
            # denominator straight off the AV matmul.
            for c in range(max(nxq, nxt)):
                if c < nxq:
                    c0, c1 = c * xchunk, min((c + 1) * xchunk, nq)
                    for s in range(c0 // qseg, c1 // qseg):
                        ss = slice(s * qseg, (s + 1) * qseg)
                        for oc in range(2):
                            ocs = slice(oc * P, (oc + 1) * P)
                            pq = psmm.tile([P, qseg], f32, tag="mm", name="pq")
                            for h in range(2):
                                nc.tensor.matmul(pq[:], wq[h][:, ocs], xq[h][:, ss],
                                                 start=(h == 0), stop=(h == 1))
                            nc.vector.tensor_copy(qT[oc][:, ss], pq[:])
                if c < nxt:
                    c0, c1 = c * xchunk, min((c + 1) * xchunk, nkv)
                    for j in range(c0 // P, c1 // P):
                        js = slice(j * P, (j + 1) * P)
                        pv = psmm.tile([P, H], f32, tag="mm", name="pv")
                        for h in range(2):
                            nc.tensor.matmul(pv[:], xt[h][:, js], wv[h][:],
                                             start=(h == 0), stop=(h == 1))
                        nc.vector.tensor_copy(vt[j][:, 0:H], pv[:])
                        nc.vector.tensor_copy(vt[j][:, H:H + 2], ones2[:])

            # ---- attention blocks ----
            # Blocks are processed in pairs: the scores matmuls for both
            # blocks of a pair share each kT stationary load (halving the
            # scores LDWEIGHTS count, which is serialized with the matmuls
            # on the PE).  Block b0's AV runs inline per key-chunk; block
            # b1's exp(S) tiles are buffered in SBUF and consumed in a
            # second AV sweep (PSUM can only hold one block's accumulators
            # plus the rotating scores tiles).
            def av_sweep(att_ps, exs, blk):
                # ic-major: each att accumulator closes after its own 1/ICH
                # of the sweep, so normalize + DMA-out overlap the rest.
                for ic in range(ICH):
                    ics = slice(ic * P, (ic + 1) * P)
                    for jc in range(JC):
                        nc.tensor.matmul(att_ps[ic][:], exs[jc][:, ics],
                                         vt[jc][:],
                                         start=(jc == 0), stop=(jc == JC - 1))

            def normalize_one(att_tile, blk, ic):
                rec = op.tile([P, 1], f32, tag="rec", name="rec")
                nc.vector.reciprocal(rec[:], att_tile[:, H:H + 1])
                ao = op.tile([P, H], f32, tag="ao", name="ao")
                nc.vector.tensor_scalar_mul(ao[:], att_tile[:, 0:H], rec[:])
                r0 = blk * iblk + ic * P
                nc.sync.dma_start(att_d[r0:r0 + P, :], ao[:])

            def normalize(att_ps, blk):
                for ic in range(ICH):
                    normalize_one(att_ps[ic], blk, ic)

            blk = 0
            GRP = 2
            while blk < NBLK:
                g = min(GRP, NBLK - blk)  # blocks in this group
                bss = [slice((blk + b) * iblk, (blk + b + 1) * iblk)
                       for b in range(g)]
                att_ps = [psatt.tile([P, H + 2], f32, tag="att", name="attps")
                          for _ in range(ICH)]
                exs = [[] for _ in range(g)]

                def av_b0(jc):
                    for ic in range(ICH):
                        ics = slice(ic * P, (ic + 1) * P)
                        nc.tensor.matmul(att_ps[ic][:], exs[0][jc][:, ics],
                                         vt[jc][:],
                                         start=(jc == 0), stop=(jc == JC - 1))

                for jc in range(JC):
                    jcs = slice(jc * P, (jc + 1) * P)
                    scs = [psmm.tile([P, iblk], f32, tag="mm", name=f"sc{b}")
                           for b in range(g)]
                    for oc in range(2):
                        for b in range(g):
                            nc.tensor.matmul(scs[b][:], xt[oc][:, jcs],
                                             qT[oc][:, bss[b]],
                                             start=(oc == 0), stop=(oc == 1))
                    for b in range(g):
                        ex = expp.tile([P, iblk], AVT, tag="ex", name=f"ex{b}")
                        nc.scalar.activation(ex[:], scs[b][:], Exp,
                                             bias=wsc[:, jc:jc + 1])
                        exs[b].append(ex)
                    av_b0(jc)
                normalize(att_ps, blk)
                for b in range(1, g):
                    att_psb = [psatt.tile([P, H + 2], f32, tag="att",
                                          name=f"attps{b}")
                               for _ in range(ICH)]
                    for ic in range(ICH):
                        ics = slice(ic * P, (ic + 1) * P)
                        for jc in range(JC):
                            nc.tensor.matmul(att_psb[ic][:],
                                             exs[b][jc][:, ics], vt[jc][:],
                                             start=(jc == 0),
                                             stop=(jc == JC - 1))
                        normalize_one(att_psb[ic], blk + b, ic)
                blk += g

    nc.compile()
    return nc


_NC_CACHE = {}


def _get_nc(nkv=N, nq=NQ, iblk=512):
    key = (nkv, nq, iblk)
    if key not in _NC_CACHE:
        _NC_CACHE[key] = build_nc(*key)
    return _NC_CACHE[key]


def _make_in_maps(x, Wq, bq, Wk, bk, Wv):
    # scores algebra: q_i.k_j = x_i (Wq^T Wk) x_j^T + u_i + w_j + c where
    # u_i and c are constant per softmax row (dropped -- softmax-invariant)
    # and w_j = x_j . (Wk^T bq) is applied as the exp() bias on-device.
    wa = np.ascontiguousarray(
        (Wq.T.astype(np.float64) @ Wk.astype(np.float64)).astype(np.float16))
    wvT = np.ascontiguousarray(Wv.T.astype(np.float16))
    wkbq = Wk.T.astype(np.float64) @ bq.astype(np.float64)
    x16 = x.astype(np.float16)
    xT = [np.ascontiguousarray(x16[b].T) for b in range(B)]
    wsc = [np.ascontiguousarray(
        (x[b].astype(np.float64) @ wkbq).astype(np.float32)
        .reshape(N // 128, 128).T) for b in range(B)]
    in_maps = []
    for c in range(8):
        b, half = c // 2, c % 2
        in_maps.append({
            "xT": xT[b],
            "xqT": np.ascontiguousarray(x16[b, half * NQ:(half + 1) * NQ, :].T),
            "wa": wa, "wvT": wvT, "wsc": wsc[b],
        })
    return in_maps


def _run(inputs, trace=False):
    from concourse.bass_utils import run_bass_kernel_spmd

    x = np.asarray(inputs["x"], dtype=np.float32)
    Wq = np.asarray(inputs["Wq"], dtype=np.float32)
    bq = np.asarray(inputs["bq"], dtype=np.float32)
    Wk = np.asarray(inputs["Wk"], dtype=np.float32)
    bk = np.asarray(inputs["bk"], dtype=np.float32)
    Wv = np.asarray(inputs["Wv"], dtype=np.float32)
    bv = np.asarray(inputs["bv"], dtype=np.float32)

    in_maps = _make_in_maps(x, Wq, bq, Wk, bk, Wv)
    # The device occasionally wedges on the first execution of a fresh
    # NEFF (NRT_EXEC_UNIT_UNRECOVERABLE); a retry with a slightly
    # perturbed program (different walrus schedule) recovers.
    last_exc = None
    for attempt in range(3):
        try:
            nc = _get_nc() if attempt == 0 else build_nc(salt=attempt)
            res = run_bass_kernel_spmd(nc, in_maps, list(range(8)), trace=trace)
            break
        except Exception as e:  # noqa: BLE001
            last_exc = e
            import os as _os
            import time as _time
            _os.environ["NEURON_RT_RESET_CORES"] = "1"
            _time.sleep(3)
    else:
        raise last_exc

    out = np.empty((B, N, H), dtype=np.float32)
    for c in range(8):
        b, half = c // 2, c % 2
        out[b, half * NQ:(half + 1) * NQ, :] = res.results[c]["att"] + bv
    return out, res


def kernel(**inputs) -> np.ndarray:
    out, _ = _run(inputs, trace=False)
    return out
